# revision 28
# baseline (speedup 1.0000x reference)
"""Trainium2 Bass kernel for nn_DBlock (StyleGAN2-style discriminator DBlock).

Reference computation (per sample, fp32):
    x = lrelu(conv3x3(y, w_conv*g3, pad=1)) * sqrt(2)            # [256,64,64]
    x = fir4x4(x, pad=2)                                         # [256,65,65]
    out = lrelu(conv3x3_s2(x, w_down*g3, pad=0)) * sqrt(2)       # [512,32,32]
    s = fir4x4_down2(y, pad=1)                                   # [256,32,32]
    s = lrelu(conv1x1(s, w_skip*g1)) * sqrt(2)                   # [512,32,32]
    return s + out

Sharding: data-parallel over batch (16 samples -> 8 cores x 2 samples),
weights replicated.

Per-core design (fp32 PSUM accumulation):
  - conv1 runs 8 of its 9 taps in fp8-e4m3 DoubleRow matmuls (both cin
    groups contracted per instruction, 2x PE throughput); the center tap
    stays fp16 for accuracy margin.  Weights are pre-scaled x64 host-side
    so the small w*g3 values sit in e4m3's normal range; the 1/64 is
    folded into conv1's activation scale.  The 4x4 FIR that follows
    conv1 low-passes the fp8 quantization noise (~3x attenuation), which
    is what makes fp8 safe here but not for conv_down/skip (kept fp16).
  - conv1 is emitted tap-outer over 2-band PSUM groups so LDWEIGHTS is
    amortized and stays hidden behind the shorter DoubleRow matmuls.
  - the separable [1,3,3,1] FIR is three 2-tap box passes per axis on the
    vector engine in fp16 (2x mode), unnormalized (x64); the 1/64 is folded
    into the activation scale of the consuming conv's PSUM drain.
  - lrelu(x)*sqrt2 == lrelu(sqrt2*x) (positive homogeneity) -> one scalar
    engine activation per PSUM tile.
  - input DMA is chunked fine-grained at the start so conv1's first PSUM
    group can start within a few us of kernel start.
"""
import sys

if "/opt/trn_rl_repo" not in sys.path:
    sys.path.insert(0, "/opt/trn_rl_repo")

import numpy as np
import ml_dtypes

import concourse.bass as bass
import concourse.tile as tile
from concourse import mybir, bacc
from concourse.bass_utils import run_bass_kernel_spmd

F32 = mybir.dt.float32
F16 = mybir.dt.float16
F8 = mybir.dt.float8e4
NP_F8 = ml_dtypes.float8_e4m3

P = 128          # partitions / channel group size
NS = 2           # samples per core
NG = 2           # cin groups (256/128)
MD = 4           # cout chunks for the 512-channel convs
SQRT2 = 1.4142135623730951
SW = 64.0        # host-side conv1 weight scale (fp8 range); folded into drain
LRELU = mybir.ActivationFunctionType.Prelu  # parametric relu: x>0 ? x : alpha*x
DR = mybir.MatmulPerfMode.DoubleRow

TAPS = [(dy, dx) for dy in range(3) for dx in range(3)]
FP8_TAPS = [t for t in TAPS if t != (1, 1)]

# input DMA row chunks: 16 rows (4KB/partition-line).  One chunk covers a
# whole 2-band conv1 group (plus halo), so both bands of a group become
# ready simultaneously and the compile-time scheduler keeps the tap-outer
# interleave (which is what hides the DoubleRow LDWEIGHTS).
IN_CHUNKS = [(16 * i, 16) for i in range(4)]

# conv1 row bands: the fp8 image rows are stored flat (66-wide, contiguous)
# so a DoubleRow matmul's moving operand is a single [128, 2, nr*66] window.
# 7 rows x 66 = 462 <= 512 PSUM columns; border columns are discarded at
# drain time.  Bands are emitted in pairs (tap-outer) to amortize LDWEIGHTS.
BANDS = [(7 * i, 7) for i in range(9)] + [(63, 1)]
BAND_GROUPS = [(0, 1), (2, 3), (4, 5), (6, 7), (8, 9)]
YW = 66          # flat fp8 image row width
Y8LEN = 1 + 66 * YW + 1  # lead pad + 66 rows + tail pad


def _build_program():
    nc = bacc.Bacc("TRN2", target_bir_lowering=False, debug=False, num_devices=8)
    lat = nc.declare_dram_parameter("lat", [NS, NG, P, 64, 64], F32, isOutput=False)
    w1 = nc.declare_dram_parameter("w1", [NG, P, 9, 256], F8, isOutput=False)
    w1c = nc.declare_dram_parameter("w1c", [P, NG, 256], F16, isOutput=False)
    wd = nc.declare_dram_parameter("wd", [NG, P, 9, 512], F16, isOutput=False)
    ws = nc.declare_dram_parameter("ws", [NG, P, 512], F16, isOutput=False)
    out = nc.declare_dram_parameter("out", [NS, MD, P, 1024], F32, isOutput=True)

    with tile.TileContext(nc) as tc:
        with (
            tc.tile_pool(name="persist", bufs=1) as pp,
            tc.tile_pool(name="stage", bufs=2) as sp,
            tc.tile_pool(name="psum1", bufs=4, space="PSUM") as ps1p,
            tc.tile_pool(name="psum2", bufs=4, space="PSUM") as psp,
            tc.tile_pool(name="ra", bufs=3) as rap,
            tc.tile_pool(name="rb", bufs=3) as rbp,
            tc.tile_pool(name="rc", bufs=2) as rcp,
        ):
            w1s8 = pp.tile([P, NG, 9, 256], F8, tag="w1s8")
            w1cs = pp.tile([P, NG, 256], F16, tag="w1cs")
            wds = pp.tile([P, NG, 9, 512], F16, tag="wds")
            wss = pp.tile([P, NG, 512], F16, tag="wss")
            # padded input image: 64x64 data at (1,1), rows 0..65, zero borders;
            # width 72 (cols 66..71 zero) so the skip FIR can read col+1 runs.
            # Double-buffered across samples so sample 1's input casts never
            # wait on conv1(0)'s reads (the DVE queue is in-order; a waiting
            # cast would block all FIR work behind it).
            ypadL = [
                pp.tile([P, NG, 66, 72], F16, name=f"ypad_{i}", tag=f"ypad_{i}")
                for i in range(NS)
            ]
            # fp8 copy of the padded image, 66-wide rows stored contiguously
            # (flat) so conv1's DoubleRow moving operand is a 1-D window; a
            # 1-element lead pad keeps the (dy=0, dx=0) tap window in-bounds.
            ypad8 = [
                pp.tile([P, NG, Y8LEN], F8, name=f"ypad8_{i}", tag=f"ypad8_{i}")
                for i in range(NS)
            ]
            # conv1 activation image: 64x64 data at (2,2), rows 0..67 (FIR pad 2),
            # width 72, zero borders.
            x1pad = pp.tile([P, NG, 68, 72], F16, tag="x1pad")
            # FIR ping-pong scratch (A also serves as the skip chain's second buf)
            A = pp.tile([P, NG, 67, 72], F16, tag="A")
            C = pp.tile([P, NG, 66, 72], F16, tag="C")
            # main FIR result (unnormalized x64), valid rows 0..64, cols 0..65;
            # double-buffered: down_and_skip(0) runs after main_fir(1) in
            # program order.
            x2 = [pp.tile([P, NG, 65, 66], F16, name=f"x2_{i}", tag=f"x2_{i}") for i in range(NS)]
            # skip FIR result (unnormalized x64), double-buffered across samples
            skipd = [pp.tile([P, NG, 32, 32], F16, name=f"skipd{i}", tag=f"skipd{i}") for i in range(NS)]

            # ---- one-time init: zero only the padding borders (vector engine;
            # gpsimd SBUF access contends with the DVE port and is slow) ----
            for i in range(NS):
                nc.vector.memset(ypadL[i][:, :, 0:1, :], 0.0)
                nc.vector.memset(ypadL[i][:, :, 65:66, :], 0.0)
                nc.vector.memset(ypadL[i][:, :, :, 0:1], 0.0)
                nc.vector.memset(ypadL[i][:, :, :, 65:72], 0.0)
                # fp8 flat image: lead pad + top border row, bottom + tail pad
                nc.vector.memset(ypad8[i][:, :, 0 : 1 + YW], 0.0)
                nc.vector.memset(ypad8[i][:, :, 1 + 65 * YW : Y8LEN], 0.0)
            nc.vector.memset(x1pad[:, :, 0:2, :], 0.0)
            nc.vector.memset(x1pad[:, :, 66:68, :], 0.0)
            nc.vector.memset(x1pad[:, :, :, 0:2], 0.0)
            nc.vector.memset(x1pad[:, :, :, 66:72], 0.0)

            def dma_w1():
                # conv1 needs these at t~0; issue on the scalar queue so they
                # run parallel to the latents DMAs on the sync queue.
                for g in range(NG):
                    nc.scalar.dma_start(w1s8[:, g], w1[g])
                nc.scalar.dma_start(w1cs[:], w1c[:])

            def dma_w2():
                # not needed until down_and_skip(0) (~100us in); issued on the
                # sync queue AFTER both samples' input chunks so this 2.6MB
                # transfer can't starve the startup input DMAs for HBM
                # bandwidth.
                for g in range(NG):
                    nc.sync.dma_start(wds[:, g], wd[g])
                    nc.sync.dma_start(wss[:, g], ws[g])

            def dma_in(n):
                # fp32 DMA staging -> fp16 interior cast (DVE 2x_2P), then a
                # full-row fp16->fp8 cast into the flat image (reads the
                # pre-zeroed border cols 0 and 65 of ypad along with the
                # data); chunked so conv1's first PSUM groups can start
                # before the whole image has landed.
                yp = ypadL[n]
                for r0, nr in IN_CHUNKS:
                    for g in range(NG):
                        s = sp.tile([P, nr, 64], F32, tag="S")
                        nc.sync.dma_start(s[:], lat[n, g, :, r0 : r0 + nr, :])
                        nc.vector.tensor_copy(
                            yp[:, g, 1 + r0 : 1 + r0 + nr, 1:65], s[:]
                        )
                        base = 1 + (1 + r0) * YW
                        nc.vector.tensor_copy(
                            ypad8[n][:, g, base : base + nr * YW],
                            yp[:, g, 1 + r0 : 1 + r0 + nr, 0:66],
                        )

            def conv1(n):
                # Row bands of 7 (PSUM [128, 7, 66], border cols discarded at
                # drain).  Emitted as 2-band groups, tap-outer, so each
                # LDWEIGHTS serves 2 matmuls.  8 taps are fp8 DoubleRow (both
                # cin groups contracted per instruction) reading flat windows
                # of ypad8; the center tap is 2 fp16 matmuls per band reading
                # full 66-wide rows of ypad so psum columns line up.
                for m in range(2):
                    ms = slice(m * P, (m + 1) * P)
                    for bA, bB in BAND_GROUPS:
                        bands = [BANDS[bA], BANDS[bB]]
                        ps = [
                            ps1p.tile(
                                [P, 7, YW], F32,
                                name=f"ps_{n}_{m}_{bA}_{b}", tag="ps1",
                            )
                            for b in range(2)
                        ]
                        kb = [0, 0]  # per-band matmul index (start/stop are per tile)
                        nper = len(FP8_TAPS) + NG  # 8 DR + 2 fp16 per band
                        for dy, dx in FP8_TAPS:
                            for b, (r0, nr) in enumerate(bands):
                                w0 = (r0 + dy) * YW + dx  # +1 base, -1 col
                                nc.tensor.matmul(
                                    ps[b][:, 0:nr, :],
                                    w1s8[:, :, 3 * dy + dx, ms],
                                    ypad8[n][:, :, w0 : w0 + nr * YW],
                                    start=(kb[b] == 0),
                                    stop=(kb[b] == nper - 1),
                                    perf_mode=DR,
                                )
                                kb[b] += 1
                        for g in range(NG):
                            for b, (r0, nr) in enumerate(bands):
                                nc.tensor.matmul(
                                    ps[b][:, 0:nr, :],
                                    w1cs[:, g, ms],
                                    ypadL[n][:, g, 1 + r0 : 1 + r0 + nr, 0:66],
                                    start=(kb[b] == 0),
                                    stop=(kb[b] == nper - 1),
                                )
                                kb[b] += 1
                        for b, (r0, nr) in enumerate(bands):
                            nc.scalar.activation(
                                x1pad[:, m, 2 + r0 : 2 + r0 + nr, 2:66],
                                ps[b][:, 0:nr, 1:65],
                                LRELU,
                                scale=SQRT2 / SW,
                                alpha=0.2,
                            )

            def skip_fir(n):
                sk = skipd[n]
                yp = ypadL[n]
                # v passes (2-tap box x3, last one row-subsampled by 2)
                nc.vector.tensor_add(C[:, :, 0:65, 0:70], yp[:, :, 0:65, 0:70], yp[:, :, 1:66, 0:70])
                nc.vector.tensor_add(A[:, :, 0:64, 0:70], C[:, :, 0:64, 0:70], C[:, :, 1:65, 0:70])
                nc.vector.tensor_add(C[:, :, 0:32, 0:70], A[:, :, 0:64:2, 0:70], A[:, :, 1:64:2, 0:70])
                # h passes
                nc.vector.tensor_add(A[:, :, 0:32, 0:68], C[:, :, 0:32, 0:68], C[:, :, 0:32, 1:69])
                nc.vector.tensor_add(C[:, :, 0:32, 0:66], A[:, :, 0:32, 0:66], A[:, :, 0:32, 1:67])
                nc.vector.tensor_add(sk[:], C[:, :, 0:32, 0:64:2], C[:, :, 0:32, 1:65:2])

            def main_fir(n):
                xx = x2[n]
                # Emitted per channel-plane so plane 0's chain can start as
                # soon as conv1(n)'s m=0 drains land, ~20us before m=1's.
                for m in range(2):
                    # v passes over full width 72
                    nc.vector.tensor_add(A[:, m, 0:67, :], x1pad[:, m, 0:67, :], x1pad[:, m, 1:68, :])
                    nc.vector.tensor_add(C[:, m, 0:66, :], A[:, m, 0:66, :], A[:, m, 1:67, :])
                    nc.vector.tensor_add(A[:, m, 0:65, :], C[:, m, 0:65, :], C[:, m, 1:66, :])
                    # h passes
                    nc.vector.tensor_add(C[:, m, 0:65, 0:70], A[:, m, 0:65, 0:70], A[:, m, 0:65, 1:71])
                    nc.vector.tensor_add(A[:, m, 0:65, 0:68], C[:, m, 0:65, 0:68], C[:, m, 0:65, 1:69])
                    nc.vector.tensor_add(xx[:, m], A[:, m, 0:65, 0:66], A[:, m, 0:65, 1:67])

            def down_and_skip(n):
                sk = skipd[n]
                xx = x2[n]
                for t in range(2):
                    r0 = 32 * t
                    for m in range(MD):
                        # skip branch: 1x1 conv over 2 cin groups
                        psk = psp.tile([P, 512], F32, tag="ps")
                        for g in range(NG):
                            nc.tensor.matmul(
                                psk[:],
                                wss[:, g, m * P : (m + 1) * P],
                                sk[:, g, 16 * t : 16 * t + 16, :],
                                start=(g == 0),
                                stop=(g == NG - 1),
                            )
                        rb = rbp.tile([P, 512], F16, tag="rb")
                        nc.scalar.activation(rb[:], psk[:], LRELU, scale=SQRT2 / 64.0, alpha=0.2)

                        # down branch: strided 3x3 conv on the FIR image
                        psd = psp.tile([P, 512], F32, tag="ps")
                        k = 0
                        for g in range(NG):
                            for dy, dx in TAPS:
                                nc.tensor.matmul(
                                    psd[:],
                                    wds[:, g, 3 * dy + dx, m * P : (m + 1) * P],
                                    xx[:, g, dy + r0 : dy + r0 + 31 : 2, dx : dx + 63 : 2],
                                    start=(k == 0),
                                    stop=(k == 17),
                                )
                                k += 1
                        ra = rap.tile([P, 512], F16, tag="ra")
                        nc.scalar.activation(ra[:], psd[:], LRELU, scale=SQRT2 / 64.0, alpha=0.2)

                        rc = rcp.tile([P, 512], F32, tag="rc")
                        nc.vector.tensor_add(rc[:], ra[:], rb[:])
                        nc.sync.dma_start(out[n, m, :, 512 * t : 512 * t + 512], rc[:])

            # ---- pipelined emission across the two samples ----
            # DVE queue order (in-order engine) is what matters here: both
            # samples' casts run first (nothing blocks them — ypad images are
            # double-buffered), then both skip FIRs (inputs ready early), then
            # the main FIRs, which chase conv1's drains plane-by-plane.  This
            # keeps main_fir(1) off the critical path of down_and_skip(1).
            dma_in(0)
            dma_w1()
            conv1(0)
            dma_in(1)
            dma_w2()
            skip_fir(0)
            skip_fir(1)
            main_fir(0)
            conv1(1)
            main_fir(1)
            down_and_skip(0)
            down_and_skip(1)

    nc.finalize()
    return nc


_PROGRAM = None


def _get_program():
    global _PROGRAM
    if _PROGRAM is None:
        _PROGRAM = _build_program()
    return _PROGRAM


def _make_in_maps(latents_in, w_conv, w_down, w_skip):
    g3 = np.float32(1.0 / np.sqrt(256 * 9))
    g1 = np.float32(1.0 / np.sqrt(256))
    lat = np.ascontiguousarray(
        np.asarray(latents_in, dtype=np.float32).reshape(8, NS, NG, P, 64, 64)
    )
    w1f = (np.asarray(w_conv, dtype=np.float32) * (g3 * np.float32(SW)))
    w1t = np.ascontiguousarray(
        w1f.transpose(1, 2, 3, 0).reshape(NG, P, 9, 256)
    ).astype(NP_F8)
    w1ct = np.ascontiguousarray(
        w1f[:, :, 1, 1].transpose(1, 0).reshape(NG, P, 256).transpose(1, 0, 2)
    ).astype(np.float16)
    wdt = np.ascontiguousarray(
        (np.asarray(w_down, dtype=np.float32) * g3).transpose(1, 2, 3, 0).reshape(NG, P, 9, 512)
    ).astype(np.float16)
    wst = np.ascontiguousarray(
        (np.asarray(w_skip, dtype=np.float32)[:, :, 0, 0] * g1).transpose(1, 0).reshape(NG, P, 512)
    ).astype(np.float16)
    return [
        {"lat": lat[i], "w1": w1t, "w1c": w1ct, "wd": wdt, "ws": wst}
        for i in range(8)
    ]


def _gather(results):
    outs = [results[i]["out"].reshape(NS, 512, 32, 32) for i in range(8)]
    return np.ascontiguousarray(np.concatenate(outs, axis=0)).astype(np.float32)


def kernel(latents_in, w_conv, w_down, w_skip):
    nc = _get_program()
    in_maps = _make_in_maps(latents_in, w_conv, w_down, w_skip)
    res = run_bass_kernel_spmd(nc, in_maps, list(range(8)))
    return _gather(res.results)


# revision 35
# speedup vs baseline: 1.0328x; 1.0328x over previous
"""Trainium2 Bass kernel for nn_DBlock (StyleGAN2-style discriminator DBlock).

Reference computation (per sample, fp32):
    x = lrelu(conv3x3(y, w_conv*g3, pad=1)) * sqrt(2)            # [256,64,64]
    x = fir4x4(x, pad=2)                                         # [256,65,65]
    out = lrelu(conv3x3_s2(x, w_down*g3, pad=0)) * sqrt(2)       # [512,32,32]
    s = fir4x4_down2(y, pad=1)                                   # [256,32,32]
    s = lrelu(conv1x1(s, w_skip*g1)) * sqrt(2)                   # [512,32,32]
    return s + out

Sharding: data-parallel over batch (16 samples -> 8 cores x 2 samples),
weights replicated.

Per-core design (fp32 PSUM accumulation):
  - conv1 runs 8 of its 9 taps in fp8-e4m3 DoubleRow matmuls (both cin
    groups contracted per instruction, 2x PE throughput); the center tap
    stays fp16 for accuracy margin.  Weights are pre-scaled x64 host-side
    so the small w*g3 values sit in e4m3's normal range; the 1/64 is
    folded into conv1's activation scale.  The 4x4 FIR that follows
    conv1 low-passes the fp8 quantization noise (~3x attenuation), which
    is what makes fp8 safe here but not for conv_down/skip (kept fp16).
  - conv1 is emitted tap-outer over 2-band PSUM groups so LDWEIGHTS is
    amortized and stays hidden behind the shorter DoubleRow matmuls.
  - the separable [1,3,3,1] FIR is three 2-tap box passes per axis on the
    vector engine in fp16 (2x mode), unnormalized (x64); the 1/64 is folded
    into the activation scale of the consuming conv's PSUM drain.
  - lrelu(x)*sqrt2 == lrelu(sqrt2*x) (positive homogeneity) -> one scalar
    engine activation per PSUM tile.
  - input DMA is chunked fine-grained at the start so conv1's first PSUM
    group can start within a few us of kernel start.
"""
import sys

if "/opt/trn_rl_repo" not in sys.path:
    sys.path.insert(0, "/opt/trn_rl_repo")

import numpy as np
import ml_dtypes

import concourse.bass as bass
import concourse.tile as tile
from concourse import mybir, bacc
from concourse.bass_utils import run_bass_kernel_spmd

F32 = mybir.dt.float32
F16 = mybir.dt.float16
F8 = mybir.dt.float8e4
NP_F8 = ml_dtypes.float8_e4m3

P = 128          # partitions / channel group size
NS = 2           # samples per core
NG = 2           # cin groups (256/128)
MD = 4           # cout chunks for the 512-channel convs
SQRT2 = 1.4142135623730951
SW = 64.0        # host-side conv1 weight scale (fp8 range); folded into drain
LRELU = mybir.ActivationFunctionType.Prelu  # parametric relu: x>0 ? x : alpha*x
DR = mybir.MatmulPerfMode.DoubleRow

TAPS = [(dy, dx) for dy in range(3) for dx in range(3)]
FP8_TAPS = [t for t in TAPS if t != (1, 1)]

# input DMA row chunks: 16 rows (4KB/partition-line).  One chunk covers a
# whole 2-band conv1 group (plus halo), so both bands of a group become
# ready simultaneously and the compile-time scheduler keeps the tap-outer
# interleave (which is what hides the DoubleRow LDWEIGHTS).
IN_CHUNKS = [(16 * i, 16) for i in range(4)]

# conv1 row bands: the fp8 image rows are stored flat (66-wide, contiguous)
# so a DoubleRow matmul's moving operand is a single [128, 2, nr*66] window.
# 7 rows x 66 = 462 <= 512 PSUM columns; border columns are discarded at
# drain time.  Bands are emitted in pairs (tap-outer) to amortize LDWEIGHTS.
BANDS = [(7 * i, 7) for i in range(9)] + [(63, 1)]
BAND_GROUPS = [(0, 1), (2, 3), (4, 5), (6, 7), (8, 9)]
YW = 66          # flat fp8 image row width
Y8LEN = 1 + 66 * YW + 1  # lead pad + 66 rows + tail pad


def _build_program():
    nc = bacc.Bacc("TRN2", target_bir_lowering=False, debug=False, num_devices=8)
    lat = nc.declare_dram_parameter("lat", [NS, NG, P, 64, 64], F32, isOutput=False)
    w1 = nc.declare_dram_parameter("w1", [NG, P, 9, 256], F8, isOutput=False)
    w1c = nc.declare_dram_parameter("w1c", [P, NG, 256], F16, isOutput=False)
    wd = nc.declare_dram_parameter("wd", [NG, P, 9, 512], F16, isOutput=False)
    ws = nc.declare_dram_parameter("ws", [NG, P, 512], F16, isOutput=False)
    out = nc.declare_dram_parameter("out", [NS, MD, P, 1024], F32, isOutput=True)

    with tile.TileContext(nc) as tc:
        with (
            tc.tile_pool(name="persist", bufs=1) as pp,
            tc.tile_pool(name="stage", bufs=2) as sp,
            tc.tile_pool(name="psum1", bufs=4, space="PSUM") as ps1p,
            tc.tile_pool(name="psum2", bufs=4, space="PSUM") as psp,
            tc.tile_pool(name="ra", bufs=8) as rap,
            tc.tile_pool(name="rb", bufs=8) as rbp,
            tc.tile_pool(name="rc", bufs=2) as rcp,
        ):
            w1s8 = pp.tile([P, NG, 9, 256], F8, tag="w1s8")
            w1cs = pp.tile([P, NG, 256], F16, tag="w1cs")
            wds = pp.tile([P, NG, 9, 512], F16, tag="wds")
            wss = pp.tile([P, NG, 512], F16, tag="wss")
            # padded input image: 64x64 data at (1,1), rows 0..65, zero borders;
            # width 72 (cols 66..71 zero) so the skip FIR can read col+1 runs.
            # Double-buffered across samples so sample 1's input casts never
            # wait on conv1(0)'s reads (the DVE queue is in-order; a waiting
            # cast would block all FIR work behind it).
            ypadL = [
                pp.tile([P, NG, 66, 72], F16, name=f"ypad_{i}", tag=f"ypad_{i}")
                for i in range(NS)
            ]
            # fp8 copy of the padded image, 66-wide rows stored contiguously
            # (flat) so conv1's DoubleRow moving operand is a 1-D window; a
            # 1-element lead pad keeps the (dy=0, dx=0) tap window in-bounds.
            ypad8 = [
                pp.tile([P, NG, Y8LEN], F8, name=f"ypad8_{i}", tag=f"ypad8_{i}")
                for i in range(NS)
            ]
            # conv1 activation image: 64x64 data at (2,2), rows 0..67 (FIR pad 2),
            # width 72, zero borders.  Double-buffered across samples so
            # conv1(1)'s drains never wait on main_fir(0)'s reads — this is
            # what keeps the PE running conv1(0) -> conv1(1) back-to-back.
            x1pad = [
                pp.tile([P, NG, 68, 72], F16, name=f"x1pad_{i}", tag=f"x1pad_{i}")
                for i in range(NS)
            ]
            # FIR ping-pong scratch, single channel-plane (the FIR chains are
            # emitted per plane and the DVE runs them serially anyway)
            A = pp.tile([P, 67, 72], F16, tag="A")
            C = pp.tile([P, 66, 72], F16, tag="C")
            # main FIR result (unnormalized x64), valid rows 0..64, cols 0..65;
            # double-buffered: down_and_skip(0) runs after main_fir(1) in
            # program order.
            x2 = [pp.tile([P, NG, 65, 66], F16, name=f"x2_{i}", tag=f"x2_{i}") for i in range(NS)]
            # skip FIR result (unnormalized x64), double-buffered across samples
            skipd = [pp.tile([P, NG, 32, 32], F16, name=f"skipd{i}", tag=f"skipd{i}") for i in range(NS)]

            # ---- one-time init: zero only the padding borders (vector engine;
            # gpsimd SBUF access contends with the DVE port and is slow) ----
            def init_borders(i):
                nc.vector.memset(ypadL[i][:, :, 0:1, :], 0.0)
                nc.vector.memset(ypadL[i][:, :, 65:66, :], 0.0)
                nc.vector.memset(ypadL[i][:, :, :, 0:1], 0.0)
                nc.vector.memset(ypadL[i][:, :, :, 65:72], 0.0)
                # fp8 flat image: lead pad + top border row, bottom + tail pad
                nc.vector.memset(ypad8[i][:, :, 0 : 1 + YW], 0.0)
                nc.vector.memset(ypad8[i][:, :, 1 + 65 * YW : Y8LEN], 0.0)
                nc.vector.memset(x1pad[i][:, :, 0:2, :], 0.0)
                nc.vector.memset(x1pad[i][:, :, 66:68, :], 0.0)
                nc.vector.memset(x1pad[i][:, :, :, 0:2], 0.0)
                nc.vector.memset(x1pad[i][:, :, :, 66:72], 0.0)

            def dma_w1():
                # conv1 needs these at t~0; issue on the scalar queue so they
                # run parallel to the latents DMAs on the sync queue.
                for g in range(NG):
                    nc.scalar.dma_start(w1s8[:, g], w1[g])
                nc.scalar.dma_start(w1cs[:], w1c[:])

            def dma_w2():
                # not needed until down_and_skip(0) (~100us in); issued on the
                # sync queue AFTER both samples' input chunks so this 2.6MB
                # transfer can't starve the startup input DMAs for HBM
                # bandwidth.
                for g in range(NG):
                    nc.sync.dma_start(wds[:, g], wd[g])
                    nc.sync.dma_start(wss[:, g], ws[g])

            def dma_in(n):
                # fp32 DMA staging -> fp16 interior cast (DVE 2x_2P), then a
                # full-row fp16->fp8 cast into the flat image (reads the
                # pre-zeroed border cols 0 and 65 of ypad along with the
                # data); chunked so conv1's first PSUM groups can start
                # before the whole image has landed.
                yp = ypadL[n]
                for r0, nr in IN_CHUNKS:
                    for g in range(NG):
                        s = sp.tile([P, nr, 64], F32, tag="S")
                        nc.sync.dma_start(s[:], lat[n, g, :, r0 : r0 + nr, :])
                        nc.vector.tensor_copy(
                            yp[:, g, 1 + r0 : 1 + r0 + nr, 1:65], s[:]
                        )
                        base = 1 + (1 + r0) * YW
                        nc.vector.tensor_copy(
                            ypad8[n][:, g, base : base + nr * YW],
                            yp[:, g, 1 + r0 : 1 + r0 + nr, 0:66],
                        )

            def conv1(n):
                # Row bands of 7 (PSUM [128, 7, 66], border cols discarded at
                # drain).  Emitted as 2-band groups, tap-outer, so each
                # LDWEIGHTS serves 2 matmuls.  8 taps are fp8 DoubleRow (both
                # cin groups contracted per instruction) reading flat windows
                # of ypad8; the center tap is 2 fp16 matmuls per band reading
                # full 66-wide rows of ypad so psum columns line up.
                for m in range(2):
                    ms = slice(m * P, (m + 1) * P)
                    for bA, bB in BAND_GROUPS:
                        bands = [BANDS[bA], BANDS[bB]]
                        ps = [
                            ps1p.tile(
                                [P, 7, YW], F32,
                                name=f"ps_{n}_{m}_{bA}_{b}", tag="ps1",
                            )
                            for b in range(2)
                        ]
                        kb = [0, 0]  # per-band matmul index (start/stop are per tile)
                        nper = len(FP8_TAPS) + NG  # 8 DR + 2 fp16 per band
                        for dy, dx in FP8_TAPS:
                            for b, (r0, nr) in enumerate(bands):
                                w0 = (r0 + dy) * YW + dx  # +1 base, -1 col
                                nc.tensor.matmul(
                                    ps[b][:, 0:nr, :],
                                    w1s8[:, :, 3 * dy + dx, ms],
                                    ypad8[n][:, :, w0 : w0 + nr * YW],
                                    start=(kb[b] == 0),
                                    stop=(kb[b] == nper - 1),
                                    perf_mode=DR,
                                )
                                kb[b] += 1
                        for g in range(NG):
                            for b, (r0, nr) in enumerate(bands):
                                nc.tensor.matmul(
                                    ps[b][:, 0:nr, :],
                                    w1cs[:, g, ms],
                                    ypadL[n][:, g, 1 + r0 : 1 + r0 + nr, 0:66],
                                    start=(kb[b] == 0),
                                    stop=(kb[b] == nper - 1),
                                )
                                kb[b] += 1
                        for b, (r0, nr) in enumerate(bands):
                            nc.scalar.activation(
                                x1pad[n][:, m, 2 + r0 : 2 + r0 + nr, 2:66],
                                ps[b][:, 0:nr, 1:65],
                                LRELU,
                                scale=SQRT2 / SW,
                                alpha=0.2,
                            )

            def skip_fir(n):
                sk = skipd[n]
                yp = ypadL[n]
                for m in range(2):
                    # v passes (2-tap box x3, last one row-subsampled by 2)
                    nc.vector.tensor_add(C[:, 0:65, 0:70], yp[:, m, 0:65, 0:70], yp[:, m, 1:66, 0:70])
                    nc.vector.tensor_add(A[:, 0:64, 0:70], C[:, 0:64, 0:70], C[:, 1:65, 0:70])
                    nc.vector.tensor_add(C[:, 0:32, 0:70], A[:, 0:64:2, 0:70], A[:, 1:64:2, 0:70])
                    # h passes
                    nc.vector.tensor_add(A[:, 0:32, 0:68], C[:, 0:32, 0:68], C[:, 0:32, 1:69])
                    nc.vector.tensor_add(C[:, 0:32, 0:66], A[:, 0:32, 0:66], A[:, 0:32, 1:67])
                    nc.vector.tensor_add(sk[:, m], C[:, 0:32, 0:64:2], C[:, 0:32, 1:65:2])

            def main_fir(n):
                xx = x2[n]
                xp = x1pad[n]
                # Emitted per channel-plane so plane 0's chain can start as
                # soon as conv1(n)'s m=0 drains land, ~20us before m=1's.
                for m in range(2):
                    # v passes over full width 72
                    nc.vector.tensor_add(A[:, 0:67, :], xp[:, m, 0:67, :], xp[:, m, 1:68, :])
                    nc.vector.tensor_add(C[:, 0:66, :], A[:, 0:66, :], A[:, 1:67, :])
                    nc.vector.tensor_add(A[:, 0:65, :], C[:, 0:65, :], C[:, 1:66, :])
                    # h passes
                    nc.vector.tensor_add(C[:, 0:65, 0:70], A[:, 0:65, 0:70], A[:, 0:65, 1:71])
                    nc.vector.tensor_add(A[:, 0:65, 0:68], C[:, 0:65, 0:68], C[:, 0:65, 1:69])
                    nc.vector.tensor_add(xx[:, m], A[:, 0:65, 0:66], A[:, 0:65, 1:67])

            def down_and_skip(n):
                sk = skipd[n]
                xx = x2[n]
                for t in range(2):
                    r0 = 32 * t
                    for m in range(MD):
                        # skip branch: 1x1 conv over 2 cin groups
                        psk = psp.tile([P, 512], F32, tag="ps")
                        for g in range(NG):
                            nc.tensor.matmul(
                                psk[:],
                                wss[:, g, m * P : (m + 1) * P],
                                sk[:, g, 16 * t : 16 * t + 16, :],
                                start=(g == 0),
                                stop=(g == NG - 1),
                            )
                        rb = rbp.tile([P, 512], F16, tag="rb")
                        nc.scalar.activation(rb[:], psk[:], LRELU, scale=SQRT2 / 64.0, alpha=0.2)

                        # down branch: strided 3x3 conv on the FIR image
                        psd = psp.tile([P, 512], F32, tag="ps")
                        k = 0
                        for g in range(NG):
                            for dy, dx in TAPS:
                                nc.tensor.matmul(
                                    psd[:],
                                    wds[:, g, 3 * dy + dx, m * P : (m + 1) * P],
                                    xx[:, g, dy + r0 : dy + r0 + 31 : 2, dx : dx + 63 : 2],
                                    start=(k == 0),
                                    stop=(k == 17),
                                )
                                k += 1
                        ra = rap.tile([P, 512], F16, tag="ra")
                        nc.scalar.activation(ra[:], psd[:], LRELU, scale=SQRT2 / 64.0, alpha=0.2)

                        rc = rcp.tile([P, 512], F32, tag="rc")
                        nc.vector.tensor_add(rc[:], ra[:], rb[:])
                        nc.sync.dma_start(out[n, m, :, 512 * t : 512 * t + 512], rc[:])

            # ---- pipelined emission across the two samples ----
            # main_fir(1) is emitted before skip_fir(1): x2[1] gates the PE
            # (down_and_skip(1)'s strided convs) while skipd[1] is only needed
            # at the same time — and the rc adds of down(0) can trail.
            init_borders(0)
            dma_in(0)
            dma_w1()
            conv1(0)
            init_borders(1)
            skip_fir(0)
            dma_in(1)
            dma_w2()
            main_fir(0)
            conv1(1)
            main_fir(1)
            skip_fir(1)
            down_and_skip(0)
            down_and_skip(1)

    nc.finalize()
    return nc


_PROGRAM = None


def _get_program():
    global _PROGRAM
    if _PROGRAM is None:
        _PROGRAM = _build_program()
    return _PROGRAM


def _make_in_maps(latents_in, w_conv, w_down, w_skip):
    g3 = np.float32(1.0 / np.sqrt(256 * 9))
    g1 = np.float32(1.0 / np.sqrt(256))
    lat = np.ascontiguousarray(
        np.asarray(latents_in, dtype=np.float32).reshape(8, NS, NG, P, 64, 64)
    )
    w1f = (np.asarray(w_conv, dtype=np.float32) * (g3 * np.float32(SW)))
    w1t = np.ascontiguousarray(
        w1f.transpose(1, 2, 3, 0).reshape(NG, P, 9, 256)
    ).astype(NP_F8)
    w1ct = np.ascontiguousarray(
        w1f[:, :, 1, 1].transpose(1, 0).reshape(NG, P, 256).transpose(1, 0, 2)
    ).astype(np.float16)
    wdt = np.ascontiguousarray(
        (np.asarray(w_down, dtype=np.float32) * g3).transpose(1, 2, 3, 0).reshape(NG, P, 9, 512)
    ).astype(np.float16)
    wst = np.ascontiguousarray(
        (np.asarray(w_skip, dtype=np.float32)[:, :, 0, 0] * g1).transpose(1, 0).reshape(NG, P, 512)
    ).astype(np.float16)
    return [
        {"lat": lat[i], "w1": w1t, "w1c": w1ct, "wd": wdt, "ws": wst}
        for i in range(8)
    ]


def _gather(results):
    outs = [results[i]["out"].reshape(NS, 512, 32, 32) for i in range(8)]
    return np.ascontiguousarray(np.concatenate(outs, axis=0)).astype(np.float32)


def kernel(latents_in, w_conv, w_down, w_skip):
    nc = _get_program()
    in_maps = _make_in_maps(latents_in, w_conv, w_down, w_skip)
    res = run_bass_kernel_spmd(nc, in_maps, list(range(8)))
    return _gather(res.results)


# revision 36
# speedup vs baseline: 1.0729x; 1.0389x over previous
"""Trainium2 Bass kernel for nn_DBlock (StyleGAN2-style discriminator DBlock).

Reference computation (per sample, fp32):
    x = lrelu(conv3x3(y, w_conv*g3, pad=1)) * sqrt(2)            # [256,64,64]
    x = fir4x4(x, pad=2)                                         # [256,65,65]
    out = lrelu(conv3x3_s2(x, w_down*g3, pad=0)) * sqrt(2)       # [512,32,32]
    s = fir4x4_down2(y, pad=1)                                   # [256,32,32]
    s = lrelu(conv1x1(s, w_skip*g1)) * sqrt(2)                   # [512,32,32]
    return s + out

Sharding: data-parallel over batch (16 samples -> 8 cores x 2 samples),
weights replicated.

Per-core design (fp32 PSUM accumulation):
  - conv1 runs 8 of its 9 taps in fp8-e4m3 DoubleRow matmuls (both cin
    groups contracted per instruction, 2x PE throughput); the center tap
    stays fp16 for accuracy margin.  Weights are pre-scaled x64 host-side
    so the small w*g3 values sit in e4m3's normal range; the 1/64 is
    folded into conv1's activation scale.  The 4x4 FIR that follows
    conv1 low-passes the fp8 quantization noise (~3x attenuation), which
    is what makes fp8 safe here but not for conv_down/skip (kept fp16).
  - conv1 is emitted tap-outer over 2-band PSUM groups so LDWEIGHTS is
    amortized and stays hidden behind the shorter DoubleRow matmuls.
  - the separable [1,3,3,1] FIR is three 2-tap box passes per axis on the
    vector engine in fp16 (2x mode), unnormalized (x64); the 1/64 is folded
    into the activation scale of the consuming conv's PSUM drain.
  - lrelu(x)*sqrt2 == lrelu(sqrt2*x) (positive homogeneity) -> one scalar
    engine activation per PSUM tile.
  - input DMA is chunked fine-grained at the start so conv1's first PSUM
    group can start within a few us of kernel start.
"""
import sys

if "/opt/trn_rl_repo" not in sys.path:
    sys.path.insert(0, "/opt/trn_rl_repo")

import numpy as np
import ml_dtypes

import concourse.bass as bass
import concourse.tile as tile
from concourse import mybir, bacc
from concourse.bass_utils import run_bass_kernel_spmd

F32 = mybir.dt.float32
F16 = mybir.dt.float16
F8 = mybir.dt.float8e4
NP_F8 = ml_dtypes.float8_e4m3

P = 128          # partitions / channel group size
NS = 2           # samples per core
NG = 2           # cin groups (256/128)
MD = 4           # cout chunks for the 512-channel convs
SQRT2 = 1.4142135623730951
SW = 64.0        # host-side conv1 weight scale (fp8 range); folded into drain
LRELU = mybir.ActivationFunctionType.Prelu  # parametric relu: x>0 ? x : alpha*x
DR = mybir.MatmulPerfMode.DoubleRow

TAPS = [(dy, dx) for dy in range(3) for dx in range(3)]
FP8_TAPS = [t for t in TAPS if t != (1, 1)]

# input DMA row chunks: 16 rows (4KB/partition-line).  One chunk covers a
# whole 2-band conv1 group (plus halo), so both bands of a group become
# ready simultaneously and the compile-time scheduler keeps the tap-outer
# interleave (which is what hides the DoubleRow LDWEIGHTS).
IN_CHUNKS = [(16 * i, 16) for i in range(4)]

# conv1 row bands: the fp8 image rows are stored flat (66-wide, contiguous)
# so a DoubleRow matmul's moving operand is a single [128, 2, nr*66] window.
# 7 rows x 66 = 462 <= 512 PSUM columns; border columns are discarded at
# drain time.  Bands are emitted in pairs (tap-outer) to amortize LDWEIGHTS.
BANDS = [(7 * i, 7) for i in range(9)] + [(63, 1)]
BAND_GROUPS = [(0, 1), (2, 3), (4, 5), (6, 7), (8, 9)]
YW = 66          # flat fp8 image row width
Y8LEN = 1 + 66 * YW + 1  # lead pad + 66 rows + tail pad


def _build_program():
    nc = bacc.Bacc("TRN2", target_bir_lowering=False, debug=False, num_devices=8)
    lat = nc.declare_dram_parameter("lat", [NS, NG, P, 64, 64], F32, isOutput=False)
    w1 = nc.declare_dram_parameter("w1", [NG, P, 9, 256], F8, isOutput=False)
    w1c = nc.declare_dram_parameter("w1c", [P, NG, 256], F16, isOutput=False)
    wd = nc.declare_dram_parameter("wd", [NG, P, 9, 512], F16, isOutput=False)
    ws = nc.declare_dram_parameter("ws", [NG, P, 512], F16, isOutput=False)
    out = nc.declare_dram_parameter("out", [NS, MD, P, 1024], F32, isOutput=True)

    with tile.TileContext(nc) as tc:
        with (
            tc.tile_pool(name="persist", bufs=1) as pp,
            tc.tile_pool(name="stage", bufs=2) as sp,
            tc.tile_pool(name="psum1", bufs=4, space="PSUM") as ps1p,
            tc.tile_pool(name="psum2", bufs=4, space="PSUM") as psp,
            tc.tile_pool(name="ra", bufs=8) as rap,
            tc.tile_pool(name="rb", bufs=8) as rbp,
            tc.tile_pool(name="rc", bufs=2) as rcp,
        ):
            w1s8 = pp.tile([P, NG, 9, 256], F8, tag="w1s8")
            w1cs = pp.tile([P, NG, 256], F16, tag="w1cs")
            wds = pp.tile([P, NG, 9, 512], F16, tag="wds")
            wss = pp.tile([P, NG, 512], F16, tag="wss")
            # padded input image: 64x64 data at (1,1), rows 0..65, zero borders;
            # width 72 (cols 66..71 zero) so the skip FIR can read col+1 runs.
            # Double-buffered across samples so sample 1's input casts never
            # wait on conv1(0)'s reads (the DVE queue is in-order; a waiting
            # cast would block all FIR work behind it).
            ypadL = [
                pp.tile([P, NG, 66, 72], F16, name=f"ypad_{i}", tag=f"ypad_{i}")
                for i in range(NS)
            ]
            # fp8 copy of the padded image, 66-wide rows stored contiguously
            # (flat) so conv1's DoubleRow moving operand is a 1-D window; a
            # 1-element lead pad keeps the (dy=0, dx=0) tap window in-bounds.
            ypad8 = [
                pp.tile([P, NG, Y8LEN], F8, name=f"ypad8_{i}", tag=f"ypad8_{i}")
                for i in range(NS)
            ]
            # conv1 activation image: 64x64 data at (2,2), rows 0..67 (FIR pad 2),
            # width 72, zero borders.  Double-buffered across samples so
            # conv1(1)'s drains never wait on main_fir(0)'s reads — this is
            # what keeps the PE running conv1(0) -> conv1(1) back-to-back.
            x1pad = [
                pp.tile([P, NG, 68, 72], F16, name=f"x1pad_{i}", tag=f"x1pad_{i}")
                for i in range(NS)
            ]
            # FIR ping-pong scratch, single channel-plane (the FIR chains are
            # emitted per plane and the DVE runs them serially anyway)
            A = pp.tile([P, 67, 72], F16, tag="A")
            C = pp.tile([P, 66, 72], F16, tag="C")
            # main FIR result (unnormalized x64), valid rows 0..64, cols 0..65;
            # double-buffered: down_and_skip(0) runs after main_fir(1) in
            # program order.
            x2 = [pp.tile([P, NG, 65, 66], F16, name=f"x2_{i}", tag=f"x2_{i}") for i in range(NS)]
            # skip FIR result (unnormalized x64), double-buffered across samples
            skipd = [pp.tile([P, NG, 32, 32], F16, name=f"skipd{i}", tag=f"skipd{i}") for i in range(NS)]

            # ---- one-time init: zero only the padding borders (vector engine;
            # gpsimd SBUF access contends with the DVE port and is slow) ----
            def init_borders(i):
                nc.gpsimd.memset(ypadL[i][:, :, 0:1, :], 0.0)
                nc.gpsimd.memset(ypadL[i][:, :, 65:66, :], 0.0)
                nc.gpsimd.memset(ypadL[i][:, :, :, 0:1], 0.0)
                nc.gpsimd.memset(ypadL[i][:, :, :, 65:72], 0.0)
                # fp8 flat image: lead pad + top border row, bottom + tail pad
                nc.gpsimd.memset(ypad8[i][:, :, 0 : 1 + YW], 0.0)
                nc.gpsimd.memset(ypad8[i][:, :, 1 + 65 * YW : Y8LEN], 0.0)
                nc.gpsimd.memset(x1pad[i][:, :, 0:2, :], 0.0)
                nc.gpsimd.memset(x1pad[i][:, :, 66:68, :], 0.0)
                nc.gpsimd.memset(x1pad[i][:, :, :, 0:2], 0.0)
                nc.gpsimd.memset(x1pad[i][:, :, :, 66:72], 0.0)

            def dma_w1():
                # conv1 needs these at t~0; issue on the scalar queue so they
                # run parallel to the latents DMAs on the sync queue.
                for g in range(NG):
                    nc.scalar.dma_start(w1s8[:, g], w1[g])
                nc.scalar.dma_start(w1cs[:], w1c[:])

            def dma_w2():
                # not needed until down_and_skip(0) (~100us in); issued on the
                # sync queue AFTER both samples' input chunks so this 2.6MB
                # transfer can't starve the startup input DMAs for HBM
                # bandwidth.
                for g in range(NG):
                    nc.sync.dma_start(wds[:, g], wd[g])
                    nc.sync.dma_start(wss[:, g], ws[g])

            def dma_in(n):
                # fp32 DMA staging -> fp16 interior cast (DVE 2x_2P), then a
                # full-row fp16->fp8 cast into the flat image (reads the
                # pre-zeroed border cols 0 and 65 of ypad along with the
                # data); chunked so conv1's first PSUM groups can start
                # before the whole image has landed.
                yp = ypadL[n]
                for r0, nr in IN_CHUNKS:
                    for g in range(NG):
                        s = sp.tile([P, nr, 64], F32, tag="S")
                        nc.sync.dma_start(s[:], lat[n, g, :, r0 : r0 + nr, :])
                        nc.vector.tensor_copy(
                            yp[:, g, 1 + r0 : 1 + r0 + nr, 1:65], s[:]
                        )
                        base = 1 + (1 + r0) * YW
                        nc.vector.tensor_copy(
                            ypad8[n][:, g, base : base + nr * YW],
                            yp[:, g, 1 + r0 : 1 + r0 + nr, 0:66],
                        )

            def conv1(n):
                # Row bands of 7 (PSUM [128, 7, 66], border cols discarded at
                # drain).  Emitted as 2-band groups, tap-outer, so each
                # LDWEIGHTS serves 2 matmuls.  8 taps are fp8 DoubleRow (both
                # cin groups contracted per instruction) reading flat windows
                # of ypad8; the center tap is 2 fp16 matmuls per band reading
                # full 66-wide rows of ypad so psum columns line up.
                for m in range(2):
                    ms = slice(m * P, (m + 1) * P)
                    for bA, bB in BAND_GROUPS:
                        bands = [BANDS[bA], BANDS[bB]]
                        ps = [
                            ps1p.tile(
                                [P, 7, YW], F32,
                                name=f"ps_{n}_{m}_{bA}_{b}", tag="ps1",
                            )
                            for b in range(2)
                        ]
                        kb = [0, 0]  # per-band matmul index (start/stop are per tile)
                        nper = len(FP8_TAPS) + NG  # 8 DR + 2 fp16 per band
                        for dy, dx in FP8_TAPS:
                            for b, (r0, nr) in enumerate(bands):
                                w0 = (r0 + dy) * YW + dx  # +1 base, -1 col
                                nc.tensor.matmul(
                                    ps[b][:, 0:nr, :],
                                    w1s8[:, :, 3 * dy + dx, ms],
                                    ypad8[n][:, :, w0 : w0 + nr * YW],
                                    start=(kb[b] == 0),
                                    stop=(kb[b] == nper - 1),
                                    perf_mode=DR,
                                )
                                kb[b] += 1
                        for g in range(NG):
                            for b, (r0, nr) in enumerate(bands):
                                nc.tensor.matmul(
                                    ps[b][:, 0:nr, :],
                                    w1cs[:, g, ms],
                                    ypadL[n][:, g, 1 + r0 : 1 + r0 + nr, 0:66],
                                    start=(kb[b] == 0),
                                    stop=(kb[b] == nper - 1),
                                )
                                kb[b] += 1
                        for b, (r0, nr) in enumerate(bands):
                            nc.scalar.activation(
                                x1pad[n][:, m, 2 + r0 : 2 + r0 + nr, 2:66],
                                ps[b][:, 0:nr, 1:65],
                                LRELU,
                                scale=SQRT2 / SW,
                                alpha=0.2,
                            )

            def skip_fir(n):
                sk = skipd[n]
                yp = ypadL[n]
                for m in range(2):
                    # v passes (2-tap box x3, last one row-subsampled by 2)
                    nc.vector.tensor_add(C[:, 0:65, 0:70], yp[:, m, 0:65, 0:70], yp[:, m, 1:66, 0:70])
                    nc.vector.tensor_add(A[:, 0:64, 0:70], C[:, 0:64, 0:70], C[:, 1:65, 0:70])
                    nc.vector.tensor_add(C[:, 0:32, 0:70], A[:, 0:64:2, 0:70], A[:, 1:64:2, 0:70])
                    # h passes
                    nc.vector.tensor_add(A[:, 0:32, 0:68], C[:, 0:32, 0:68], C[:, 0:32, 1:69])
                    nc.vector.tensor_add(C[:, 0:32, 0:66], A[:, 0:32, 0:66], A[:, 0:32, 1:67])
                    nc.vector.tensor_add(sk[:, m], C[:, 0:32, 0:64:2], C[:, 0:32, 1:65:2])

            def main_fir(n):
                xx = x2[n]
                xp = x1pad[n]
                # Emitted per channel-plane so plane 0's chain can start as
                # soon as conv1(n)'s m=0 drains land, ~20us before m=1's.
                for m in range(2):
                    # v passes over full width 72
                    nc.vector.tensor_add(A[:, 0:67, :], xp[:, m, 0:67, :], xp[:, m, 1:68, :])
                    nc.vector.tensor_add(C[:, 0:66, :], A[:, 0:66, :], A[:, 1:67, :])
                    nc.vector.tensor_add(A[:, 0:65, :], C[:, 0:65, :], C[:, 1:66, :])
                    # h passes
                    nc.vector.tensor_add(C[:, 0:65, 0:70], A[:, 0:65, 0:70], A[:, 0:65, 1:71])
                    nc.vector.tensor_add(A[:, 0:65, 0:68], C[:, 0:65, 0:68], C[:, 0:65, 1:69])
                    nc.vector.tensor_add(xx[:, m], A[:, 0:65, 0:66], A[:, 0:65, 1:67])

            def down_and_skip(n):
                sk = skipd[n]
                xx = x2[n]
                for t in range(2):
                    r0 = 32 * t
                    for m in range(MD):
                        # skip branch: 1x1 conv over 2 cin groups
                        psk = psp.tile([P, 512], F32, tag="ps")
                        for g in range(NG):
                            nc.tensor.matmul(
                                psk[:],
                                wss[:, g, m * P : (m + 1) * P],
                                sk[:, g, 16 * t : 16 * t + 16, :],
                                start=(g == 0),
                                stop=(g == NG - 1),
                            )
                        rb = rbp.tile([P, 512], F16, tag="rb")
                        nc.scalar.activation(rb[:], psk[:], LRELU, scale=SQRT2 / 64.0, alpha=0.2)

                        # down branch: strided 3x3 conv on the FIR image
                        psd = psp.tile([P, 512], F32, tag="ps")
                        k = 0
                        for g in range(NG):
                            for dy, dx in TAPS:
                                nc.tensor.matmul(
                                    psd[:],
                                    wds[:, g, 3 * dy + dx, m * P : (m + 1) * P],
                                    xx[:, g, dy + r0 : dy + r0 + 31 : 2, dx : dx + 63 : 2],
                                    start=(k == 0),
                                    stop=(k == 17),
                                )
                                k += 1
                        ra = rap.tile([P, 512], F16, tag="ra")
                        nc.scalar.activation(ra[:], psd[:], LRELU, scale=SQRT2 / 64.0, alpha=0.2)

                        rc = rcp.tile([P, 512], F32, tag="rc")
                        # final add on gpsimd: keeps the vector engine free
                        # for the FIR chains, which gate the down convs
                        nc.gpsimd.tensor_add(rc[:], ra[:], rb[:])
                        nc.sync.dma_start(out[n, m, :, 512 * t : 512 * t + 512], rc[:])

            # ---- pipelined emission across the two samples ----
            # main_fir(1) is emitted before skip_fir(1): x2[1] gates the PE
            # (down_and_skip(1)'s strided convs) while skipd[1] is only needed
            # at the same time — and the rc adds of down(0) can trail.
            init_borders(0)
            dma_in(0)
            dma_w1()
            conv1(0)
            init_borders(1)
            skip_fir(0)
            dma_in(1)
            dma_w2()
            skip_fir(1)
            main_fir(0)
            conv1(1)
            main_fir(1)
            down_and_skip(0)
            down_and_skip(1)

    nc.finalize()
    return nc


_PROGRAM = None


def _get_program():
    global _PROGRAM
    if _PROGRAM is None:
        _PROGRAM = _build_program()
    return _PROGRAM


def _make_in_maps(latents_in, w_conv, w_down, w_skip):
    g3 = np.float32(1.0 / np.sqrt(256 * 9))
    g1 = np.float32(1.0 / np.sqrt(256))
    lat = np.ascontiguousarray(
        np.asarray(latents_in, dtype=np.float32).reshape(8, NS, NG, P, 64, 64)
    )
    w1f = (np.asarray(w_conv, dtype=np.float32) * (g3 * np.float32(SW)))
    w1t = np.ascontiguousarray(
        w1f.transpose(1, 2, 3, 0).reshape(NG, P, 9, 256)
    ).astype(NP_F8)
    w1ct = np.ascontiguousarray(
        w1f[:, :, 1, 1].transpose(1, 0).reshape(NG, P, 256).transpose(1, 0, 2)
    ).astype(np.float16)
    wdt = np.ascontiguousarray(
        (np.asarray(w_down, dtype=np.float32) * g3).transpose(1, 2, 3, 0).reshape(NG, P, 9, 512)
    ).astype(np.float16)
    wst = np.ascontiguousarray(
        (np.asarray(w_skip, dtype=np.float32)[:, :, 0, 0] * g1).transpose(1, 0).reshape(NG, P, 512)
    ).astype(np.float16)
    return [
        {"lat": lat[i], "w1": w1t, "w1c": w1ct, "wd": wdt, "ws": wst}
        for i in range(8)
    ]


def _gather(results):
    outs = [results[i]["out"].reshape(NS, 512, 32, 32) for i in range(8)]
    return np.ascontiguousarray(np.concatenate(outs, axis=0)).astype(np.float32)


def kernel(latents_in, w_conv, w_down, w_skip):
    nc = _get_program()
    in_maps = _make_in_maps(latents_in, w_conv, w_down, w_skip)
    res = run_bass_kernel_spmd(nc, in_maps, list(range(8)))
    return _gather(res.results)
